# revision 37
# baseline (speedup 1.0000x reference)
"""Bidirectional GRU encoder (Keras GRUCell reset_after=True) on Trainium2.

Problem shapes (hardcoded): V=32000, E=512, U=1024, B=32, T=256.

Strategy
--------
The time recurrence is strictly sequential and its per-step cost is dominated
by streaming U_r (1024x3072) through the PE — independent of batch size, so
batch sharding buys nothing. Instead, each core runs BOTH directions
interleaved step-by-step: the two dependency chains are independent, so one
direction's matmuls fill the PE while the other runs its gate elementwise
chain — this both hides the serial latency and keeps the PE's HAM clock-gate
warm (idle >3.4us throttles the PE to 1.2GHz). All 8 cores run the identical
program (the harness reads core 0's outputs).

Everything on-chip lives in a "transposed" layout with the gate/hidden dim on
partitions (KT=8 chunks of U, GT=24 tiles of 3U):

  hT   [128, KT*B]   hT[p, 32k+b]  = h[b, 128k+p]
  G    [128, GT*B]   G[p, 32j+b]   = (h @ U_r)[b, 128j+p]
  xwT  same layout, precomputed x @ W per step

so every gate op runs at full 128-partition width and the updated hT is
directly the next step's matmul operand (no transposes in the loop).

Per step, h @ U_r is computed in two stages: stage 1 packs 8 K-chunks into
the PE's four 32-column groups (tile_position col-tiling -> concurrent
streams) producing 4 partial sums on partition groups; stage 2 multiplies
by a stacked identity [I32;I32;I32;I32] which simultaneously reduces the
partials and transposes into the gate layout.

The input projection xW = emb[x] @ W is computed on-device in blocks of
TBLK=8 steps per direction, double buffered in SBUF, inside the same For_i
loop — it rides in leftover PE slots and never round-trips DRAM.

Matmuls run in bf16 with fp32 PSUM accumulation.
"""

import numpy as np

V, E, U, B, T = 32000, 512, 1024, 32, 256
G = 3 * U            # 3072 gate width (z|r|n)
KT = U // 128        # 8  k-chunks of the hidden dim
GT = G // 128        # 24 g-tiles of the gate dim
ET = E // 128        # 4  e-chunks of the embedding dim
TBLK = 8             # recurrence steps per xW block (256 tokens)
TOKB = TBLK * B      # 256 tokens per block
YB = 4               # steps per y-output DMA group
QB = TBLK // YB      # y-groups per block

N_CORES = 8
C = KT * B           # 256 columns of an hT/gate-third tile


def build_program(t_total=T):
    import concourse.bacc as bacc
    import concourse.bass as bass
    import concourse.mybir as mybir
    import concourse.tile as tile
    from concourse.bass import ds
    from concourse.masks import make_identity

    nblk = t_total // TBLK
    assert nblk >= 2 and nblk % 2 == 0

    fp32 = mybir.dt.float32
    bf16 = mybir.dt.bfloat16
    i32 = mybir.dt.int32
    AF = mybir.ActivationFunctionType
    OP = mybir.AluOpType

    # Bacc (not raw Bass): its compile() pass splits multi-sem waits into
    # EventSemaphore chains — walrus accepts only ONE sync wait per inst.
    nc = bacc.Bacc("TRN2")

    emb_d = nc.dram_tensor("emb", [V, E], fp32, kind="ExternalInput")
    s4i_d = nc.dram_tensor("s4i", [128, 32], fp32, kind="ExternalInput")

    dirs = ("f", "b")
    din = {}
    for dn in dirs:
        din[dn] = dict(
            x=nc.dram_tensor(f"x_{dn}", [t_total, B, 1], i32, kind="ExternalInput"),
            w=nc.dram_tensor(f"w_{dn}", [E, G], bf16, kind="ExternalInput"),
            ur=nc.dram_tensor(f"ur_{dn}", [U, G], bf16, kind="ExternalInput"),
            b0pg=nc.dram_tensor(f"b0pg_{dn}", [128, GT], fp32, kind="ExternalInput"),
            b1n=nc.dram_tensor(f"b1n_{dn}", [128, C], fp32, kind="ExternalInput"),
            h0t=nc.dram_tensor(f"h0t_{dn}", [128, C], fp32, kind="ExternalInput"),
            y=nc.dram_tensor(f"y_{dn}", [t_total // YB, 128, YB * C], fp32,
                             kind="ExternalOutput"),
            h=nc.dram_tensor(f"h_{dn}", [128, C], fp32, kind="ExternalOutput"),
        )

    with tile.TileContext(nc) as tc:
        with (
            tc.tile_pool(name="const", bufs=1) as cpool,
            tc.tile_pool(name="work", bufs=2) as wpool,
            tc.tile_pool(name="psum", bufs=1, space="PSUM") as ppool,
        ):
            s4i = cpool.tile([128, 32], bf16, name="s4i")
            ident = cpool.tile([128, 128], bf16, name="ident")
            make_identity(nc, ident[:, :])
            s4stg = wpool.tile([128, 32], fp32, tag="s4stg", bufs=1)
            nc.sync.dma_start(out=s4stg[:, :], in_=s4i_d[:, :])
            nc.vector.tensor_copy(s4i[:, :], s4stg[:, :])

            ds_state = {}
            for dn in dirs:
                st = {}
                st["in"] = din[dn]
                st["ur_sb"] = cpool.tile([128, KT * G], bf16, name=f"ur_{dn}_sb")
                st["h_sb"] = cpool.tile([128, C], bf16, name=f"h_{dn}_sb")
                st["b0pg"] = cpool.tile([128, GT], fp32, name=f"b0pg_{dn}_sb")
                st["b1n"] = cpool.tile([128, C], fp32, name=f"b1n_{dn}_sb")
                st["xw"] = [
                    cpool.tile([128, TBLK * GT * B], bf16, name=f"xw{w}_{dn}")
                    for w in range(2)
                ]
                st["yblk"] = [None]
                st["last_hf"] = None
                ds_state[dn] = st

                for k in range(KT):
                    nc.sync.dma_start(
                        out=st["ur_sb"][:, G * k:G * (k + 1)],
                        in_=st["in"]["ur"][128 * k:128 * (k + 1), :])
                nc.sync.dma_start(out=st["b0pg"][:, :], in_=st["in"]["b0pg"][:, :])
                nc.sync.dma_start(out=st["b1n"][:, :], in_=st["in"]["b1n"][:, :])
                h0stg = wpool.tile([128, C], fp32, tag="h0stg", bufs=2)
                nc.sync.dma_start(out=h0stg[:, :], in_=st["in"]["h0t"][:, :])
                nc.vector.tensor_copy(st["h_sb"][:, :], h0stg[:, :])

            # ------------------------------------------------------------------
            def phase1_block(st, t0, xw):
                """xW^T for steps [t0, t0+TBLK) of one direction -> xw tile.
                xw col = 768*t_local + 32*j + b. t0 may be reg-affine."""
                xin = st["in"]
                wtile = wpool.tile([128, ET * G], bf16, tag="wst", bufs=1,
                                   name="wtile")
                for e in range(ET):
                    nc.sync.dma_start(
                        out=wtile[:, G * e:G * (e + 1)],
                        in_=xin["w"][128 * e:128 * (e + 1), :],
                    )
                xet = wpool.tile([128, ET * TOKB], bf16, tag="xet", bufs=2)
                for i in range(TOKB // 128):  # 2 tok-tiles of 128 tokens
                    idx = wpool.tile([128, 1], i32, tag="idx", bufs=2)
                    nc.gpsimd.dma_start(
                        out=idx[:, :], in_=xin["x"][ds(t0 + 4 * i, 4), :, :])
                    # bounce via Pool compute: collapses the indirect DMA's
                    # deps (idx ready + WAR on xe, prev reader also Pool) to
                    # ONE Pool sem — dynamic DMAs fit a single wait
                    idx2 = wpool.tile([128, 1], i32, tag="idx2", bufs=2)
                    nc.gpsimd.tensor_copy(idx2[:, :], idx[:, :])
                    xe = wpool.tile([128, E], fp32, tag="xe", bufs=2)
                    nc.gpsimd.indirect_dma_start(
                        out=xe[:, :],
                        out_offset=None,
                        in_=emb_d[:, :],
                        in_offset=bass.IndirectOffsetOnAxis(ap=idx2[:, :1], axis=0),
                    )
                    # cast on Pool so the PE transpose sees ONE producer (the
                    # LDWEIGHTS slot also fits a single wait)
                    xeb = wpool.tile([128, E], bf16, tag="xeb", bufs=2)
                    nc.gpsimd.tensor_copy(xeb[:, :], xe[:, :])
                    for e in range(ET):
                        tp = ppool.tile([128, 128], bf16, tag="p1ps", bufs=2)
                        nc.tensor.transpose(
                            out=tp[:, :], in_=xeb[:, 128 * e:128 * (e + 1)],
                            identity=ident[:, :],
                        )
                        nc.vector.tensor_copy(
                            xet[:, TOKB * e + 128 * i: TOKB * e + 128 * (i + 1)],
                            tp[:, :],
                        )
                for j in range(GT):
                    ps = ppool.tile([128, TOKB], fp32, tag="p1ps", bufs=2)
                    for e in range(ET):
                        nc.tensor.matmul(
                            ps[:, :],
                            lhsT=wtile[:, G * e + 128 * j: G * e + 128 * (j + 1)],
                            rhs=xet[:, TOKB * e: TOKB * (e + 1)],
                            start=(e == 0), stop=(e == ET - 1),
                        )
                    src = ps[:, :].rearrange("p (t b) -> p t b", b=B)
                    dst = xw[:, :].rearrange(
                        "p (t g b) -> p t g b", g=GT, b=B)[:, :, j, :]
                    if j % 2 == 0:
                        nc.vector.tensor_scalar_add(dst, src, st["b0pg"][:, j:j + 1])
                    else:
                        nc.scalar.activation(
                            dst, src, AF.Identity, bias=st["b0pg"][:, j:j + 1])

            # ------------------------------------------------------------------
            def step(st, qbase, xw, s):
                """One recurrence step of one direction; consumes xw slab s,
                updates h_sb, stages y per YB steps. qbase = base time // YB."""
                xw0 = (GT * B) * s
                h_sb = st["h_sb"]

                gps = ppool.tile([128, GT * B], fp32, tag="g_ps", bufs=2,
                                 name="gps")
                H = G // 3  # thirds of the gate dim (PSUM budget)
                for hh in range(3):
                    pps = ppool.tile([128, H], fp32, tag="p_ps", bufs=1,
                                     name="pps")
                    for cg in range(4):
                        for kk in range(2):
                            k = 2 * cg + kk
                            for n in range(H // 512):  # 2
                                c0 = G * k + H * hh + 512 * n
                                nc.tensor.matmul(
                                    pps[32 * cg:32 * (cg + 1), 512 * n:512 * (n + 1)],
                                    lhsT=h_sb[:, 32 * k:32 * (k + 1)],
                                    rhs=st["ur_sb"][:, c0:c0 + 512],
                                    start=(kk == 0), stop=(kk == 1),
                                    tile_position=(0, 32 * cg),
                                )
                    pb = wpool.tile([128, H], bf16, tag="pb", bufs=3, name="pb")
                    nc.vector.tensor_copy(pb[:, 0:H // 2], pps[:, 0:H // 2])
                    nc.scalar.copy(pb[:, H // 2:H], pps[:, H // 2:H])
                    for j in range(GT // 3):  # 8
                        jj = (GT // 3) * hh + j
                        nc.tensor.matmul(
                            gps[:, 32 * jj:32 * (jj + 1)],
                            lhsT=pb[:, 128 * j:128 * (j + 1)],
                            rhs=s4i[:, :],
                            start=True, stop=True,
                        )

                # ---- gates: cols [0:C)=z [C:2C)=r [2C:3C)=n ----
                zr = wpool.tile([128, 2 * C], bf16, tag="zr", bufs=2, name="zr")
                nc.vector.tensor_tensor(
                    out=zr[:, :], in0=gps[:, 0:2 * C],
                    in1=xw[:, xw0:xw0 + 2 * C], op=OP.add)
                zt = wpool.tile([128, C], bf16, tag="zt", bufs=2, name="zt")
                rt = wpool.tile([128, C], bf16, tag="rt", bufs=2, name="rt")
                nc.scalar.activation(zt[:, :], zr[:, 0:C], AF.Sigmoid)
                nc.scalar.activation(rt[:, :], zr[:, C:2 * C], AF.Sigmoid)
                hnb = wpool.tile([128, C], bf16, tag="hnb", bufs=2, name="hnb")
                nc.vector.tensor_tensor(
                    out=hnb[:, :], in0=gps[:, 2 * C:3 * C], in1=st["b1n"][:, :],
                    op=OP.add)
                t2 = wpool.tile([128, C], bf16, tag="t2", bufs=2, name="t2")
                nc.vector.tensor_mul(t2[:, :], rt[:, :], hnb[:, :])
                t3 = wpool.tile([128, C], bf16, tag="t3", bufs=2, name="t3")
                nc.vector.tensor_tensor(
                    out=t3[:, :], in0=t2[:, :],
                    in1=xw[:, xw0 + 2 * C:xw0 + 3 * C], op=OP.add)
                nt = wpool.tile([128, C], bf16, tag="nt", bufs=2, name="nt")
                nc.scalar.activation(nt[:, :], t3[:, :], AF.Tanh)
                dt_ = wpool.tile([128, C], bf16, tag="dt", bufs=2, name="dt")
                nc.vector.tensor_sub(dt_[:, :], h_sb[:, :], nt[:, :])
                hf = wpool.tile([128, C], bf16, tag="hf", bufs=2, name="hf")
                nc.vector.tensor_mul(hf[:, :], zt[:, :], dt_[:, :])
                if s % YB == 0:
                    st["yblk"][0] = wpool.tile(
                        [128, YB * C], fp32, tag="yblk", bufs=2, name="yblk")
                yblk = st["yblk"][0]
                hf2 = yblk[:, (s % YB) * C:(s % YB + 1) * C]
                nc.vector.tensor_tensor(out=hf2, in0=hf[:, :], in1=nt[:, :],
                                        op=OP.add)
                nc.vector.tensor_copy(h_sb[:, :], hf2)
                if s % YB == YB - 1:
                    dma_eng = [nc.sync, nc.scalar][(s // YB) % 2]
                    dma_eng.dma_start(
                        out=st["in"]["y"][ds(qbase + s // YB, 1), :, :],
                        in_=yblk[:, :])
                st["last_hf"] = hf2

            # ------------------------------------------------------------------
            F, Bd = ds_state["f"], ds_state["b"]

            # prologue: blocks 0,1 for both dirs
            for st in (F, Bd):
                phase1_block(st, 0, st["xw"][0])
            for st in (F, Bd):
                phase1_block(st, TBLK, st["xw"][1])

            if nblk > 2:
                with tc.For_i(0, (nblk - 2) * QB, 2 * QB) as q0:
                    for s in range(TBLK):
                        step(F, q0, F["xw"][0], s)
                        step(Bd, q0, Bd["xw"][0], s)
                    phase1_block(F, q0 * YB + 2 * TBLK, F["xw"][0])
                    phase1_block(Bd, q0 * YB + 2 * TBLK, Bd["xw"][0])
                    for s in range(TBLK):
                        step(F, q0 + QB, F["xw"][1], s)
                        step(Bd, q0 + QB, Bd["xw"][1], s)
                    phase1_block(F, q0 * YB + 3 * TBLK, F["xw"][1])
                    phase1_block(Bd, q0 * YB + 3 * TBLK, Bd["xw"][1])
            te = (nblk - 2) * TBLK
            for s in range(TBLK):
                step(F, te // YB, F["xw"][0], s)
                step(Bd, te // YB, Bd["xw"][0], s)
            for s in range(TBLK):
                step(F, te // YB + QB, F["xw"][1], s)
                step(Bd, te // YB + QB, Bd["xw"][1], s)

            nc.sync.dma_start(out=din["f"]["h"][:, :], in_=F["last_hf"])
            nc.sync.dma_start(out=din["b"]["h"][:, :], in_=Bd["last_hf"])

    nc.finalize()
    return nc


# ----------------------------------------------------------------------------
# host-side packing / unpacking
# ----------------------------------------------------------------------------

def _pack_dir(x_tb, w, ur, b, h0):
    import ml_dtypes
    b = np.asarray(b, np.float32)
    b0, b1 = b[0], b[1]
    badd = b0 + np.where(np.arange(G) < 2 * U, b1, 0.0)
    b0pg = np.ascontiguousarray(badd.reshape(GT, 128).T)
    b1n = np.ascontiguousarray(np.broadcast_to(
        b1[2 * U:].reshape(KT, 128).T[:, :, None], (128, KT, B)
    ).reshape(128, KT * B))
    h0t = np.ascontiguousarray(
        np.asarray(h0, np.float32).reshape(B, KT, 128).transpose(2, 1, 0)
    ).reshape(128, KT * B)
    return dict(
        x=np.ascontiguousarray(x_tb, np.int32).reshape(T, B, 1),
        w=np.asarray(w, np.float32).astype(ml_dtypes.bfloat16),
        ur=np.asarray(ur, np.float32).astype(ml_dtypes.bfloat16),
        b0pg=b0pg, b1n=b1n, h0t=h0t,
    )


def _unpack_y(y_raw):
    """y_raw [T//4, 128, 4*KT*B] -> y [B, T, U];
    y_raw[q, p, 256*s + 32*k + b] = y[b, 4q+s, 128k+p]."""
    return np.ascontiguousarray(
        y_raw.reshape(T // YB, 128, YB, KT, B).transpose(4, 0, 2, 3, 1)
    ).reshape(B, T, U)


def _unpack_h(h_raw):
    return np.ascontiguousarray(
        h_raw.reshape(128, KT, B).transpose(2, 1, 0)
    ).reshape(B, U)


_CACHED = {}


def _get_program():
    if "nc" not in _CACHED:
        _CACHED["nc"] = build_program()
    return _CACHED["nc"]


def kernel(x, emb, W_f, U_f, b_f, W_b, U_b, b_b, h0_f, h0_b, _trace=False):
    from concourse.bass_utils import run_bass_kernel_spmd

    x = np.asarray(x)
    im = {"emb": np.asarray(emb, np.float32),
          "s4i": np.tile(np.eye(32, dtype=np.float32), (4, 1))}
    for dn, (xd, w, ur, b, h0) in {
        "f": (x.T, W_f, U_f, b_f, h0_f),
        "b": (x.T[::-1], W_b, U_b, b_b, h0_b),
    }.items():
        for k, v in _pack_dir(xd, w, ur, b, h0).items():
            im[f"{k}_{dn}"] = v

    nc = _get_program()
    res = run_bass_kernel_spmd(
        nc, [im] * N_CORES, core_ids=list(range(N_CORES)), trace=_trace,
    )
    r0 = res.results[0]
    y_f = _unpack_y(r0["y_f"])
    y_b = _unpack_y(r0["y_b"])[:, ::-1]
    h_f = _unpack_h(r0["h_f"])
    h_b = _unpack_h(r0["h_b"])
    y = np.concatenate([y_f, y_b], axis=-1)
    if _trace:
        kernel.last_exec_ns = res.exec_time_ns
        kernel.last_results = res
    return (np.ascontiguousarray(y, np.float32), h_f.astype(np.float32),
            h_b.astype(np.float32))


# revision 38
# speedup vs baseline: 1.5561x; 1.5561x over previous
"""Bidirectional GRU encoder (Keras GRUCell reset_after=True) on Trainium2.

Problem shapes (hardcoded): V=32000, E=512, U=1024, B=32, T=256.

Strategy
--------
The time recurrence is strictly sequential and its per-step cost is dominated
by streaming U_r (1024x3072) through the PE — independent of batch size, so
batch sharding buys nothing. Instead, each core runs BOTH directions
interleaved step-by-step: the two dependency chains are independent, so one
direction's matmuls fill the PE while the other runs its gate elementwise
chain — this both hides the serial latency and keeps the PE's HAM clock-gate
warm (idle >3.4us throttles the PE to 1.2GHz). All 8 cores run the identical
program (the harness reads core 0's outputs).

Everything on-chip lives in a "transposed" layout with the gate/hidden dim on
partitions (KT=8 chunks of U, GT=24 tiles of 3U):

  hT   [128, KT*B]   hT[p, 32k+b]  = h[b, 128k+p]
  G    [128, GT*B]   G[p, 32j+b]   = (h @ U_r)[b, 128j+p]
  xwT  same layout, precomputed x @ W per step

so every gate op runs at full 128-partition width and the updated hT is
directly the next step's matmul operand (no transposes in the loop).

Per step, h @ U_r is computed in two stages: stage 1 packs 8 K-chunks into
the PE's four 32-column groups (tile_position col-tiling -> concurrent
streams) producing 4 partial sums on partition groups; stage 2 multiplies
by a stacked identity [I32;I32;I32;I32] which simultaneously reduces the
partials and transposes into the gate layout.

The input projection xW = emb[x] @ W is computed on-device in blocks of
TBLK=8 steps per direction, double buffered in SBUF, inside the same For_i
loop — it rides in leftover PE slots and never round-trips DRAM.

Matmuls run in bf16 with fp32 PSUM accumulation.
"""

import numpy as np

V, E, U, B, T = 32000, 512, 1024, 32, 256
G = 3 * U            # 3072 gate width (z|r|n)
KT = U // 128        # 8  k-chunks of the hidden dim
GT = G // 128        # 24 g-tiles of the gate dim
ET = E // 128        # 4  e-chunks of the embedding dim
TBLK = 8             # recurrence steps per xW block (256 tokens)
TOKB = TBLK * B      # 256 tokens per block
YB = 4               # steps per y-output DMA group
QB = TBLK // YB      # y-groups per block

N_CORES = 8
C = KT * B           # 256 columns of an hT/gate-third tile


def build_program(t_total=T):
    import concourse.bacc as bacc
    import concourse.bass as bass
    import concourse.mybir as mybir
    import concourse.tile as tile
    from concourse.bass import ds
    from concourse.masks import make_identity

    nblk = t_total // TBLK
    assert nblk >= 2 and nblk % 2 == 0

    fp32 = mybir.dt.float32
    bf16 = mybir.dt.bfloat16
    i32 = mybir.dt.int32
    AF = mybir.ActivationFunctionType
    OP = mybir.AluOpType

    # Bacc (not raw Bass): its compile() pass splits multi-sem waits into
    # EventSemaphore chains — walrus accepts only ONE sync wait per inst.
    nc = bacc.Bacc("TRN2")

    emb_d = nc.dram_tensor("emb", [V, E], fp32, kind="ExternalInput")
    s4i_d = nc.dram_tensor("s4i", [128, 32], fp32, kind="ExternalInput")

    dirs = ("f", "b")
    din = {}
    for dn in dirs:
        din[dn] = dict(
            x=nc.dram_tensor(f"x_{dn}", [t_total, B, 1], i32, kind="ExternalInput"),
            w=nc.dram_tensor(f"w_{dn}", [E, G], bf16, kind="ExternalInput"),
            ur=nc.dram_tensor(f"ur_{dn}", [U, G], bf16, kind="ExternalInput"),
            b0pg=nc.dram_tensor(f"b0pg_{dn}", [128, GT], fp32, kind="ExternalInput"),
            b1n=nc.dram_tensor(f"b1n_{dn}", [128, C], fp32, kind="ExternalInput"),
            h0t=nc.dram_tensor(f"h0t_{dn}", [128, C], fp32, kind="ExternalInput"),
            y=nc.dram_tensor(f"y_{dn}", [t_total // YB, 128, YB * C], fp32,
                             kind="ExternalOutput"),
            h=nc.dram_tensor(f"h_{dn}", [128, C], fp32, kind="ExternalOutput"),
        )

    with tile.TileContext(nc) as tc:
        with (
            tc.tile_pool(name="const", bufs=1) as cpool,
            tc.tile_pool(name="work", bufs=2) as wpool,
            tc.tile_pool(name="psum", bufs=1, space="PSUM") as ppool,
        ):
            s4i = cpool.tile([128, 32], bf16, name="s4i")
            ident = cpool.tile([128, 128], bf16, name="ident")
            make_identity(nc, ident[:, :])
            s4stg = wpool.tile([128, 32], fp32, tag="s4stg", bufs=1)
            nc.sync.dma_start(out=s4stg[:, :], in_=s4i_d[:, :])
            nc.vector.tensor_copy(s4i[:, :], s4stg[:, :])

            ds_state = {}
            for dn in dirs:
                st = {}
                st["in"] = din[dn]
                st["ur_sb"] = cpool.tile([128, KT * G], bf16, name=f"ur_{dn}_sb")
                st["h_sb"] = cpool.tile([128, C], bf16, name=f"h_{dn}_sb")
                st["b0pg"] = cpool.tile([128, GT], fp32, name=f"b0pg_{dn}_sb")
                st["b1n"] = cpool.tile([128, C], fp32, name=f"b1n_{dn}_sb")
                st["xw"] = [
                    cpool.tile([128, TBLK * GT * B], bf16, name=f"xw{w}_{dn}")
                    for w in range(2)
                ]
                st["yblk"] = [None]
                st["last_hf"] = None
                ds_state[dn] = st

                for k in range(KT):
                    nc.sync.dma_start(
                        out=st["ur_sb"][:, G * k:G * (k + 1)],
                        in_=st["in"]["ur"][128 * k:128 * (k + 1), :])
                nc.sync.dma_start(out=st["b0pg"][:, :], in_=st["in"]["b0pg"][:, :])
                nc.sync.dma_start(out=st["b1n"][:, :], in_=st["in"]["b1n"][:, :])
                h0stg = wpool.tile([128, C], fp32, tag="h0stg", bufs=2)
                nc.sync.dma_start(out=h0stg[:, :], in_=st["in"]["h0t"][:, :])
                nc.vector.tensor_copy(st["h_sb"][:, :], h0stg[:, :])

            # ------------------------------------------------------------------
            def phase1_block(st, t0, xw):
                """xW^T for steps [t0, t0+TBLK) of one direction -> xw tile.
                xw col = 768*t_local + 32*j + b. t0 may be reg-affine."""
                xin = st["in"]
                wtile = wpool.tile([128, ET * G], bf16, tag="wst", bufs=1,
                                   name="wtile")
                for e in range(ET):
                    nc.sync.dma_start(
                        out=wtile[:, G * e:G * (e + 1)],
                        in_=xin["w"][128 * e:128 * (e + 1), :],
                    )
                xet = wpool.tile([128, ET * TOKB], bf16, tag="xet", bufs=2)
                for i in range(TOKB // 128):  # 2 tok-tiles of 128 tokens
                    idx = wpool.tile([128, 1], i32, tag="idx", bufs=2)
                    nc.gpsimd.dma_start(
                        out=idx[:, :], in_=xin["x"][ds(t0 + 4 * i, 4), :, :])
                    # bounce via Pool compute: collapses the indirect DMA's
                    # deps (idx ready + WAR on xe, prev reader also Pool) to
                    # ONE Pool sem — dynamic DMAs fit a single wait
                    idx2 = wpool.tile([128, 1], i32, tag="idx2", bufs=2)
                    nc.gpsimd.tensor_copy(idx2[:, :], idx[:, :])
                    xe = wpool.tile([128, E], fp32, tag="xe", bufs=2)
                    nc.gpsimd.indirect_dma_start(
                        out=xe[:, :],
                        out_offset=None,
                        in_=emb_d[:, :],
                        in_offset=bass.IndirectOffsetOnAxis(ap=idx2[:, :1], axis=0),
                    )
                    # cast on Pool so the PE transpose sees ONE producer (the
                    # LDWEIGHTS slot also fits a single wait)
                    xeb = wpool.tile([128, E], bf16, tag="xeb", bufs=2)
                    nc.gpsimd.tensor_copy(xeb[:, :], xe[:, :])
                    for e in range(ET):
                        tp = ppool.tile([128, 128], bf16, tag="p1ps", bufs=2)
                        nc.tensor.transpose(
                            out=tp[:, :], in_=xeb[:, 128 * e:128 * (e + 1)],
                            identity=ident[:, :],
                        )
                        nc.vector.tensor_copy(
                            xet[:, TOKB * e + 128 * i: TOKB * e + 128 * (i + 1)],
                            tp[:, :],
                        )
                for j in range(GT):
                    ps = ppool.tile([128, TOKB], fp32, tag="p1ps", bufs=2)
                    for e in range(ET):
                        nc.tensor.matmul(
                            ps[:, :],
                            lhsT=wtile[:, G * e + 128 * j: G * e + 128 * (j + 1)],
                            rhs=xet[:, TOKB * e: TOKB * (e + 1)],
                            start=(e == 0), stop=(e == ET - 1),
                        )
                    src = ps[:, :].rearrange("p (t b) -> p t b", b=B)
                    dst = xw[:, :].rearrange(
                        "p (t g b) -> p t g b", g=GT, b=B)[:, :, j, :]
                    if j % 2 == 0:
                        nc.vector.tensor_scalar_add(dst, src, st["b0pg"][:, j:j + 1])
                    else:
                        nc.scalar.activation(
                            dst, src, AF.Identity, bias=st["b0pg"][:, j:j + 1])

            # ------------------------------------------------------------------
            def step(st, qbase, xw, s):
                """One recurrence step of one direction; consumes xw slab s,
                updates h_sb, stages y per YB steps. qbase = base time // YB."""
                xw0 = (GT * B) * s
                h_sb = st["h_sb"]

                # direct transposed matmul: G[:, j-tile] = sum_k U_r[k,j].T @ hT[k]
                # 24x8 (LDWEIGHTS + N=32 MM) pairs; with bf16 FWL these retire
                # every ~25-55ns once the PE queue is deep, and there is no
                # intermediate PSUM juggling to serialize the two directions
                gps = ppool.tile([128, GT * B], fp32, tag="g_ps", bufs=2,
                                 name="gps")
                for j in range(GT):
                    for k in range(KT):
                        nc.tensor.matmul(
                            gps[:, 32 * j:32 * (j + 1)],
                            lhsT=st["ur_sb"][:, G * k + 128 * j: G * k + 128 * (j + 1)],
                            rhs=h_sb[:, 32 * k:32 * (k + 1)],
                            start=(k == 0), stop=(k == KT - 1),
                        )

                # ---- gates: cols [0:C)=z [C:2C)=r [2C:3C)=n ----
                zr = wpool.tile([128, 2 * C], bf16, tag="zr", bufs=2, name="zr")
                nc.vector.tensor_tensor(
                    out=zr[:, :], in0=gps[:, 0:2 * C],
                    in1=xw[:, xw0:xw0 + 2 * C], op=OP.add)
                zt = wpool.tile([128, C], bf16, tag="zt", bufs=2, name="zt")
                rt = wpool.tile([128, C], bf16, tag="rt", bufs=2, name="rt")
                nc.scalar.activation(zt[:, :], zr[:, 0:C], AF.Sigmoid)
                nc.scalar.activation(rt[:, :], zr[:, C:2 * C], AF.Sigmoid)
                hnb = wpool.tile([128, C], bf16, tag="hnb", bufs=2, name="hnb")
                nc.vector.tensor_tensor(
                    out=hnb[:, :], in0=gps[:, 2 * C:3 * C], in1=st["b1n"][:, :],
                    op=OP.add)
                t2 = wpool.tile([128, C], bf16, tag="t2", bufs=2, name="t2")
                nc.vector.tensor_mul(t2[:, :], rt[:, :], hnb[:, :])
                t3 = wpool.tile([128, C], bf16, tag="t3", bufs=2, name="t3")
                nc.vector.tensor_tensor(
                    out=t3[:, :], in0=t2[:, :],
                    in1=xw[:, xw0 + 2 * C:xw0 + 3 * C], op=OP.add)
                nt = wpool.tile([128, C], bf16, tag="nt", bufs=2, name="nt")
                nc.scalar.activation(nt[:, :], t3[:, :], AF.Tanh)
                dt_ = wpool.tile([128, C], bf16, tag="dt", bufs=2, name="dt")
                nc.vector.tensor_sub(dt_[:, :], h_sb[:, :], nt[:, :])
                hf = wpool.tile([128, C], bf16, tag="hf", bufs=2, name="hf")
                nc.vector.tensor_mul(hf[:, :], zt[:, :], dt_[:, :])
                if s % YB == 0:
                    st["yblk"][0] = wpool.tile(
                        [128, YB * C], fp32, tag="yblk", bufs=2, name="yblk")
                yblk = st["yblk"][0]
                hf2 = yblk[:, (s % YB) * C:(s % YB + 1) * C]
                nc.vector.tensor_tensor(out=hf2, in0=hf[:, :], in1=nt[:, :],
                                        op=OP.add)
                nc.vector.tensor_copy(h_sb[:, :], hf2)
                if s % YB == YB - 1:
                    dma_eng = [nc.sync, nc.scalar][(s // YB) % 2]
                    dma_eng.dma_start(
                        out=st["in"]["y"][ds(qbase + s // YB, 1), :, :],
                        in_=yblk[:, :])
                st["last_hf"] = hf2

            # ------------------------------------------------------------------
            F, Bd = ds_state["f"], ds_state["b"]

            # prologue: blocks 0,1 for both dirs
            for st in (F, Bd):
                phase1_block(st, 0, st["xw"][0])
            for st in (F, Bd):
                phase1_block(st, TBLK, st["xw"][1])

            if nblk > 2:
                with tc.For_i(0, (nblk - 2) * QB, 2 * QB) as q0:
                    for s in range(TBLK):
                        step(F, q0, F["xw"][0], s)
                        step(Bd, q0, Bd["xw"][0], s)
                    phase1_block(F, q0 * YB + 2 * TBLK, F["xw"][0])
                    phase1_block(Bd, q0 * YB + 2 * TBLK, Bd["xw"][0])
                    for s in range(TBLK):
                        step(F, q0 + QB, F["xw"][1], s)
                        step(Bd, q0 + QB, Bd["xw"][1], s)
                    phase1_block(F, q0 * YB + 3 * TBLK, F["xw"][1])
                    phase1_block(Bd, q0 * YB + 3 * TBLK, Bd["xw"][1])
            te = (nblk - 2) * TBLK
            for s in range(TBLK):
                step(F, te // YB, F["xw"][0], s)
                step(Bd, te // YB, Bd["xw"][0], s)
            for s in range(TBLK):
                step(F, te // YB + QB, F["xw"][1], s)
                step(Bd, te // YB + QB, Bd["xw"][1], s)

            nc.sync.dma_start(out=din["f"]["h"][:, :], in_=F["last_hf"])
            nc.sync.dma_start(out=din["b"]["h"][:, :], in_=Bd["last_hf"])

    nc.finalize()
    return nc


# ----------------------------------------------------------------------------
# host-side packing / unpacking
# ----------------------------------------------------------------------------

def _pack_dir(x_tb, w, ur, b, h0):
    import ml_dtypes
    b = np.asarray(b, np.float32)
    b0, b1 = b[0], b[1]
    badd = b0 + np.where(np.arange(G) < 2 * U, b1, 0.0)
    b0pg = np.ascontiguousarray(badd.reshape(GT, 128).T)
    b1n = np.ascontiguousarray(np.broadcast_to(
        b1[2 * U:].reshape(KT, 128).T[:, :, None], (128, KT, B)
    ).reshape(128, KT * B))
    h0t = np.ascontiguousarray(
        np.asarray(h0, np.float32).reshape(B, KT, 128).transpose(2, 1, 0)
    ).reshape(128, KT * B)
    return dict(
        x=np.ascontiguousarray(x_tb, np.int32).reshape(T, B, 1),
        w=np.asarray(w, np.float32).astype(ml_dtypes.bfloat16),
        ur=np.asarray(ur, np.float32).astype(ml_dtypes.bfloat16),
        b0pg=b0pg, b1n=b1n, h0t=h0t,
    )


def _unpack_y(y_raw):
    """y_raw [T//4, 128, 4*KT*B] -> y [B, T, U];
    y_raw[q, p, 256*s + 32*k + b] = y[b, 4q+s, 128k+p]."""
    return np.ascontiguousarray(
        y_raw.reshape(T // YB, 128, YB, KT, B).transpose(4, 0, 2, 3, 1)
    ).reshape(B, T, U)


def _unpack_h(h_raw):
    return np.ascontiguousarray(
        h_raw.reshape(128, KT, B).transpose(2, 1, 0)
    ).reshape(B, U)


_CACHED = {}


def _get_program():
    if "nc" not in _CACHED:
        _CACHED["nc"] = build_program()
    return _CACHED["nc"]


def kernel(x, emb, W_f, U_f, b_f, W_b, U_b, b_b, h0_f, h0_b, _trace=False):
    from concourse.bass_utils import run_bass_kernel_spmd

    x = np.asarray(x)
    im = {"emb": np.asarray(emb, np.float32),
          "s4i": np.tile(np.eye(32, dtype=np.float32), (4, 1))}
    for dn, (xd, w, ur, b, h0) in {
        "f": (x.T, W_f, U_f, b_f, h0_f),
        "b": (x.T[::-1], W_b, U_b, b_b, h0_b),
    }.items():
        for k, v in _pack_dir(xd, w, ur, b, h0).items():
            im[f"{k}_{dn}"] = v

    nc = _get_program()
    res = run_bass_kernel_spmd(
        nc, [im] * N_CORES, core_ids=list(range(N_CORES)), trace=_trace,
    )
    r0 = res.results[0]
    y_f = _unpack_y(r0["y_f"])
    y_b = _unpack_y(r0["y_b"])[:, ::-1]
    h_f = _unpack_h(r0["h_f"])
    h_b = _unpack_h(r0["h_b"])
    y = np.concatenate([y_f, y_b], axis=-1)
    if _trace:
        kernel.last_exec_ns = res.exec_time_ns
        kernel.last_results = res
    return (np.ascontiguousarray(y, np.float32), h_f.astype(np.float32),
            h_b.astype(np.float32))


# revision 43
# speedup vs baseline: 2.9878x; 1.9200x over previous
"""Bidirectional GRU encoder (Keras GRUCell reset_after=True) on Trainium2.

Problem shapes (hardcoded): V=32000, E=512, U=1024, B=32, T=256.

Strategy
--------
The time recurrence is strictly sequential and its per-step cost is dominated
by feeding U_r (1024x3072 weights) into the PE every step — independent of
batch size, so batch sharding buys nothing. The two directions instead run on
different cores (SPMD: one program; a core's *data* selects its direction —
even cores get forward inputs, odd cores get time-reversed inputs). The
harness reads core 0 (forward) and core 1 (backward).

Everything on-chip lives in a "transposed" layout with the gate/hidden dim on
partitions (KT=8 chunks of U, GT=24 tiles of 3U):

  hT   [128, KT*B]   hT[p, 32k+b]  = h[b, 128k+p]
  G    [128, GT*B]   G[p, 32j+b]   = gates[b, 128j+p]
  xwT  same layout, precomputed x @ W per step

so every gate op runs at full 128-partition width and the updated hT is
directly the next step's matmul operand (no transposes in the loop).

The recurrent matmul is 24x8 (LDWEIGHTS + N=32 MM) pairs with U_r tiles
stationary; with bf16 FWL these retire every ~34ns back-to-back. The
precomputed xw_zr slab and the recurrent n-gate bias are then ACCUMULATED
into the same PSUM with two identity matmuls, so sigmoid/tanh inputs come
straight out of PSUM and the DVE chain stays short.

The input projection xW = emb[x] @ W is computed on-device in blocks of
TBLK=16 steps (512 tokens), double buffered in SBUF inside the same For_i
loop — it rides in leftover PE slots and never round-trips DRAM, and it
keeps the PE's HAM clock-gate warm during the gate chain.

Matmuls run in bf16 with fp32 PSUM accumulation.
"""

import numpy as np

V, E, U, B, T = 32000, 512, 1024, 32, 256
G = 3 * U            # 3072 gate width (z|r|n)
KT = U // 128        # 8  k-chunks of the hidden dim
GT = G // 128        # 24 g-tiles of the gate dim
ET = E // 128        # 4  e-chunks of the embedding dim
TBLK = 16            # recurrence steps per xW block (512 tokens)
TOKB = TBLK * B      # 512 tokens per block
YB = 4               # steps per y-output DMA group
QB = TBLK // YB      # y-groups per block

N_CORES = 8
C = KT * B           # 256 columns of an hT/gate-third tile


def build_program(t_total=T):
    import concourse.bacc as bacc
    import concourse.bass as bass
    import concourse.mybir as mybir
    import concourse.tile as tile
    from concourse.bass import ds
    from concourse.masks import make_identity

    nblk = t_total // TBLK
    assert nblk >= 2 and nblk % 2 == 0

    fp32 = mybir.dt.float32
    bf16 = mybir.dt.bfloat16
    i32 = mybir.dt.int32
    AF = mybir.ActivationFunctionType
    OP = mybir.AluOpType

    # Bacc (not raw Bass): its compile() pass splits multi-sem waits into
    # EventSemaphore chains — walrus accepts only ONE sync wait per inst.
    nc = bacc.Bacc("TRN2")

    x_d = nc.dram_tensor("x_ids", [t_total, B, 1], i32, kind="ExternalInput")
    emb_d = nc.dram_tensor("emb", [V, E], fp32, kind="ExternalInput")
    w_d = nc.dram_tensor("w", [E, G], bf16, kind="ExternalInput")
    ur_d = nc.dram_tensor("ur", [U, G], bf16, kind="ExternalInput")
    b0pg_d = nc.dram_tensor("b0pg", [128, GT], fp32, kind="ExternalInput")
    b1n_d = nc.dram_tensor("b1n", [128, C], fp32, kind="ExternalInput")
    h0t_d = nc.dram_tensor("h0t", [128, C], fp32, kind="ExternalInput")
    y_d = nc.dram_tensor("y_out", [t_total // YB, 128, YB * C], fp32,
                         kind="ExternalOutput")
    h_d = nc.dram_tensor("h_out", [128, C], fp32, kind="ExternalOutput")

    with tile.TileContext(nc) as tc:
        with (
            tc.tile_pool(name="const", bufs=1) as cpool,
            tc.tile_pool(name="work", bufs=2) as wpool,
            tc.tile_pool(name="psum", bufs=1, space="PSUM") as ppool,
        ):
            ur_sb = cpool.tile([128, KT * G], bf16, name="ur_sb")
            w_sb = cpool.tile([128, ET * G], bf16, name="w_sb")
            xw_ab = [cpool.tile([128, TBLK * GT * B], bf16, name=f"xw{w}")
                     for w in range(2)]
            h_sb = cpool.tile([128, C], bf16, name="h_sb")
            b0pg = cpool.tile([128, GT], fp32, name="b0pg")
            b1n_bf = cpool.tile([128, C], bf16, name="b1n_bf")
            ident = cpool.tile([128, 128], bf16, name="ident")
            make_identity(nc, ident[:, :])

            for k in range(KT):
                nc.sync.dma_start(out=ur_sb[:, G * k:G * (k + 1)],
                                  in_=ur_d[128 * k:128 * (k + 1), :])
            for e in range(ET):
                nc.sync.dma_start(out=w_sb[:, G * e:G * (e + 1)],
                                  in_=w_d[128 * e:128 * (e + 1), :])
            nc.sync.dma_start(out=b0pg[:, :], in_=b0pg_d[:, :])
            bstg = wpool.tile([128, C], fp32, tag="bstg", bufs=1)
            nc.sync.dma_start(out=bstg[:, :], in_=b1n_d[:, :])
            nc.vector.tensor_copy(b1n_bf[:, :], bstg[:, :])
            h0stg = wpool.tile([128, C], fp32, tag="h0stg", bufs=1)
            nc.sync.dma_start(out=h0stg[:, :], in_=h0t_d[:, :])
            nc.vector.tensor_copy(h_sb[:, :], h0stg[:, :])

            # ------------------------------------------------------------------
            def phase1_block(t0, xw):
                """xW^T for steps [t0, t0+TBLK) -> xw (col = 768*t + 32*j + b).
                t0 may be reg-affine."""
                xet = wpool.tile([128, ET * TOKB], bf16, tag="xet", bufs=2)
                for i in range(TOKB // 128):  # 4 tok-tiles
                    idx = wpool.tile([128, 1], i32, tag="idx", bufs=2)
                    nc.gpsimd.dma_start(out=idx[:, :],
                                        in_=x_d[ds(t0 + 4 * i, 4), :, :])
                    # bounce via Pool compute: collapses the indirect DMA's
                    # deps (idx ready + WAR on xe, prev reader also Pool) to
                    # ONE Pool sem — dynamic DMAs fit a single sem wait
                    idx2 = wpool.tile([128, 1], i32, tag="idx2", bufs=2)
                    nc.gpsimd.tensor_copy(idx2[:, :], idx[:, :])
                    xe = wpool.tile([128, E], fp32, tag="xe", bufs=2)
                    nc.gpsimd.indirect_dma_start(
                        out=xe[:, :], out_offset=None, in_=emb_d[:, :],
                        in_offset=bass.IndirectOffsetOnAxis(ap=idx2[:, :1], axis=0))
                    # cast on Pool so the PE transpose sees ONE producer (the
                    # LDWEIGHTS slot also fits a single sem wait)
                    xeb = wpool.tile([128, E], bf16, tag="xeb", bufs=2)
                    nc.gpsimd.tensor_copy(xeb[:, :], xe[:, :])
                    for e in range(ET):
                        tp = ppool.tile([128, 128], bf16, tag="p1ps", bufs=2)
                        nc.tensor.transpose(
                            out=tp[:, :], in_=xeb[:, 128 * e:128 * (e + 1)],
                            identity=ident[:, :])
                        nc.vector.tensor_copy(
                            xet[:, TOKB * e + 128 * i: TOKB * e + 128 * (i + 1)],
                            tp[:, :])
                for j in range(GT):
                    ps = ppool.tile([128, TOKB], fp32, tag="p1ps", bufs=2)
                    for e in range(ET):
                        nc.tensor.matmul(
                            ps[:, :],
                            lhsT=w_sb[:, G * e + 128 * j: G * e + 128 * (j + 1)],
                            rhs=xet[:, TOKB * e: TOKB * (e + 1)],
                            start=(e == 0), stop=(e == ET - 1))
                    src = ps[:, :].rearrange("p (t b) -> p t b", b=B)
                    dst = xw[:, :].rearrange(
                        "p (t g b) -> p t g b", g=GT, b=B)[:, :, j, :]
                    if j % 2 == 0:
                        nc.vector.tensor_scalar_add(dst, src, b0pg[:, j:j + 1])
                    else:
                        nc.scalar.activation(dst, src, AF.Identity,
                                             bias=b0pg[:, j:j + 1])

            # ------------------------------------------------------------------
            yblk_cur = [None]
            last_hf = [None]

            def step(qbase, xw, s):
                """One recurrence step; consumes xw slab s, updates h_sb,
                stages y per YB steps. qbase = (time of slab 0) // YB."""
                xw0 = (GT * B) * s

                # G = h @ U_r via 24x8 (LDW + N=32 MM) pairs (~34ns each).
                # An identity matmul OPENS each PSUM region preloaded with the
                # precomputed [xw_z|xw_r] (resp. recurrent bias b1_n), and the
                # U_r matmuls accumulate on top — so the nonlinearity inputs
                # come straight from PSUM with no DVE adds.
                zr_ps = ppool.tile([128, 2 * C], fp32, tag="zr_ps", bufs=2,
                                   name="zr_ps")
                n_ps = ppool.tile([128, C], fp32, tag="n_ps", bufs=2,
                                  name="n_ps")
                nc.tensor.matmul(
                    zr_ps[:, :], lhsT=ident[:, :], rhs=xw[:, xw0:xw0 + 2 * C],
                    start=True, stop=False, skip_group_check=True)
                for j in range(16):
                    for k in range(KT):
                        nc.tensor.matmul(
                            zr_ps[:, 32 * j:32 * (j + 1)],
                            lhsT=ur_sb[:, G * k + 128 * j: G * k + 128 * (j + 1)],
                            rhs=h_sb[:, 32 * k:32 * (k + 1)],
                            start=False, stop=(j == 15 and k == KT - 1),
                            skip_group_check=True)
                nc.tensor.matmul(
                    n_ps[:, :], lhsT=ident[:, :], rhs=b1n_bf[:, :],
                    start=True, stop=False, skip_group_check=True)
                for j in range(16, GT):
                    for k in range(KT):
                        nc.tensor.matmul(
                            n_ps[:, 32 * (j - 16):32 * (j - 15)],
                            lhsT=ur_sb[:, G * k + 128 * j: G * k + 128 * (j + 1)],
                            rhs=h_sb[:, 32 * k:32 * (k + 1)],
                            start=False, stop=(j == GT - 1 and k == KT - 1),
                            skip_group_check=True)

                # ---- gates ----
                zt = wpool.tile([128, C], bf16, tag="zt", bufs=2, name="zt")
                rt = wpool.tile([128, C], bf16, tag="rt", bufs=2, name="rt")
                nc.scalar.activation(zt[:, :], zr_ps[:, 0:C], AF.Sigmoid)
                nc.scalar.activation(rt[:, :], zr_ps[:, C:2 * C], AF.Sigmoid)
                t2 = wpool.tile([128, C], bf16, tag="t2", bufs=2, name="t2")
                nc.vector.tensor_tensor(
                    out=t2[:, :], in0=n_ps[:, :], in1=rt[:, :],
                    op=OP.mult)
                t3 = wpool.tile([128, C], bf16, tag="t3", bufs=2, name="t3")
                nc.vector.tensor_tensor(
                    out=t3[:, :], in0=t2[:, :],
                    in1=xw[:, xw0 + 2 * C:xw0 + 3 * C], op=OP.add)
                nt = wpool.tile([128, C], bf16, tag="nt", bufs=2, name="nt")
                nc.scalar.activation(nt[:, :], t3[:, :], AF.Tanh)
                dt_ = wpool.tile([128, C], bf16, tag="dt", bufs=2, name="dt")
                nc.vector.tensor_sub(dt_[:, :], h_sb[:, :], nt[:, :])
                hf = wpool.tile([128, C], bf16, tag="hf", bufs=2, name="hf")
                nc.vector.tensor_mul(hf[:, :], zt[:, :], dt_[:, :])
                if s % YB == 0:
                    yblk_cur[0] = wpool.tile([128, YB * C], fp32, tag="yblk",
                                             bufs=2, name="yblk")
                yblk = yblk_cur[0]
                hf2 = yblk[:, (s % YB) * C:(s % YB + 1) * C]
                nc.vector.tensor_tensor(out=hf2, in0=hf[:, :], in1=nt[:, :],
                                        op=OP.add)
                nc.vector.tensor_copy(h_sb[:, :], hf2)
                if s % YB == YB - 1:
                    dma_eng = [nc.sync, nc.scalar][(s // YB) % 2]
                    dma_eng.dma_start(out=y_d[ds(qbase + s // YB, 1), :, :],
                                      in_=yblk[:, :])
                last_hf[0] = hf2

            # ------------------------------------------------------------------
            phase1_block(0, xw_ab[0])
            phase1_block(TBLK, xw_ab[1])

            if nblk > 2:
                with tc.For_i(0, (nblk - 2) * QB, 2 * QB) as q0:
                    for s in range(TBLK):
                        step(q0, xw_ab[0], s)
                    phase1_block(q0 * YB + 2 * TBLK, xw_ab[0])
                    for s in range(TBLK):
                        step(q0 + QB, xw_ab[1], s)
                    phase1_block(q0 * YB + 3 * TBLK, xw_ab[1])
            te = (nblk - 2) * TBLK
            for s in range(TBLK):
                step(te // YB, xw_ab[0], s)
            for s in range(TBLK):
                step(te // YB + QB, xw_ab[1], s)

            nc.sync.dma_start(out=h_d[:, :], in_=last_hf[0])

    nc.finalize()
    return nc


# ----------------------------------------------------------------------------
# host-side packing / unpacking
# ----------------------------------------------------------------------------

def _pack_dir(x_tb, w, ur, b, h0, emb):
    import ml_dtypes
    b = np.asarray(b, np.float32)
    b0, b1 = b[0], b[1]
    badd = b0 + np.where(np.arange(G) < 2 * U, b1, 0.0)
    b0pg = np.ascontiguousarray(badd.reshape(GT, 128).T)
    b1n = np.ascontiguousarray(np.broadcast_to(
        b1[2 * U:].reshape(KT, 128).T[:, :, None], (128, KT, B)
    ).reshape(128, KT * B))
    h0t = np.ascontiguousarray(
        np.asarray(h0, np.float32).reshape(B, KT, 128).transpose(2, 1, 0)
    ).reshape(128, KT * B)
    return {
        "x_ids": np.ascontiguousarray(x_tb, np.int32).reshape(T, B, 1),
        "emb": np.asarray(emb, np.float32),
        "w": np.asarray(w, np.float32).astype(ml_dtypes.bfloat16),
        "ur": np.asarray(ur, np.float32).astype(ml_dtypes.bfloat16),
        "b0pg": b0pg, "b1n": b1n, "h0t": h0t,
    }


def _unpack_y(y_raw):
    """y_raw [T//4, 128, 4*KT*B] -> y [B, T, U];
    y_raw[q, p, 256*s + 32*k + b] = y[b, 4q+s, 128k+p]."""
    return np.ascontiguousarray(
        y_raw.reshape(T // YB, 128, YB, KT, B).transpose(4, 0, 2, 3, 1)
    ).reshape(B, T, U)


def _unpack_h(h_raw):
    return np.ascontiguousarray(
        h_raw.reshape(128, KT, B).transpose(2, 1, 0)
    ).reshape(B, U)


_CACHED = {}


def _get_program():
    if "nc" not in _CACHED:
        _CACHED["nc"] = build_program()
    return _CACHED["nc"]


def kernel(x, emb, W_f, U_f, b_f, W_b, U_b, b_b, h0_f, h0_b, _trace=False):
    from concourse.bass_utils import run_bass_kernel_spmd

    x = np.asarray(x)
    im_f = _pack_dir(x.T, W_f, U_f, b_f, h0_f, emb)
    im_b = _pack_dir(x.T[::-1], W_b, U_b, b_b, h0_b, emb)

    nc = _get_program()
    res = run_bass_kernel_spmd(
        nc, [im_f if c % 2 == 0 else im_b for c in range(N_CORES)],
        core_ids=list(range(N_CORES)), trace=_trace,
    )
    rf, rb = res.results[0], res.results[1]
    y_f = _unpack_y(rf["y_out"])
    y_b = _unpack_y(rb["y_out"])[:, ::-1]
    h_f = _unpack_h(rf["h_out"])
    h_b = _unpack_h(rb["h_out"])
    y = np.concatenate([y_f, y_b], axis=-1)
    if _trace:
        kernel.last_exec_ns = res.exec_time_ns
        kernel.last_results = res
    return (np.ascontiguousarray(y, np.float32), h_f.astype(np.float32),
            h_b.astype(np.float32))


# revision 47
# speedup vs baseline: 3.1591x; 1.0573x over previous
"""Bidirectional GRU encoder (Keras GRUCell reset_after=True) on Trainium2.

Problem shapes (hardcoded): V=32000, E=512, U=1024, B=32, T=256.

Strategy
--------
The time recurrence is strictly sequential and its per-step cost is dominated
by feeding U_r (1024x3072 weights) into the PE every step — independent of
batch size, so batch sharding buys nothing. The two directions instead run on
different cores (SPMD: one program; a core's *data* selects its direction —
even cores get forward inputs, odd cores get time-reversed inputs). The
harness reads core 0 (forward) and core 1 (backward).

Everything on-chip lives in a "transposed" layout with the gate/hidden dim on
partitions (KT=8 chunks of U, GT=24 tiles of 3U):

  hT   [128, KT*B]   hT[p, 32k+b]  = h[b, 128k+p]
  G    [128, GT*B]   G[p, 32j+b]   = gates[b, 128j+p]
  xwT  same layout, precomputed x @ W per step

so every gate op runs at full 128-partition width and the updated hT is
directly the next step's matmul operand (no transposes in the loop).

The recurrent matmul is 24x8 (LDWEIGHTS + N=32 MM) pairs with U_r tiles
stationary; with bf16 FWL these retire every ~34ns back-to-back. The
precomputed xw_zr slab and the recurrent n-gate bias are then ACCUMULATED
into the same PSUM with two identity matmuls, so sigmoid/tanh inputs come
straight out of PSUM and the DVE chain stays short.

The input projection xW = emb[x] @ W is computed on-device in blocks of
TBLK=16 steps (512 tokens), double buffered in SBUF inside the same For_i
loop — it rides in leftover PE slots and never round-trips DRAM, and it
keeps the PE's HAM clock-gate warm during the gate chain.

Matmuls run in bf16 with fp32 PSUM accumulation.
"""

import numpy as np

V, E, U, B, T = 32000, 512, 1024, 32, 256
G = 3 * U            # 3072 gate width (z|r|n)
KT = U // 128        # 8  k-chunks of the hidden dim
GT = G // 128        # 24 g-tiles of the gate dim
ET = E // 128        # 4  e-chunks of the embedding dim
TBLK = 16            # recurrence steps per xW block (512 tokens)
TOKB = TBLK * B      # 512 tokens per block
YB = 4               # steps per y-output DMA group
QB = TBLK // YB      # y-groups per block

N_CORES = 8
C = KT * B           # 256 columns of an hT/gate-third tile


def build_program(t_total=T):
    import concourse.bacc as bacc
    import concourse.bass as bass
    import concourse.mybir as mybir
    import concourse.tile as tile
    from concourse.bass import ds
    from concourse.masks import make_identity

    nblk = t_total // TBLK
    assert nblk >= 2 and nblk % 2 == 0

    fp32 = mybir.dt.float32
    bf16 = mybir.dt.bfloat16
    i32 = mybir.dt.int32
    AF = mybir.ActivationFunctionType
    OP = mybir.AluOpType

    # Bacc (not raw Bass): its compile() pass splits multi-sem waits into
    # EventSemaphore chains — walrus accepts only ONE sync wait per inst.
    nc = bacc.Bacc("TRN2")

    x_d = nc.dram_tensor("x_ids", [t_total, B, 1], i32, kind="ExternalInput")
    emb_d = nc.dram_tensor("emb", [V, E], fp32, kind="ExternalInput")
    w_d = nc.dram_tensor("w", [E, G], bf16, kind="ExternalInput")
    ur_d = nc.dram_tensor("ur", [U, G], bf16, kind="ExternalInput")
    b0pg_d = nc.dram_tensor("b0pg", [128, GT], fp32, kind="ExternalInput")
    b1n_d = nc.dram_tensor("b1n", [128, C], fp32, kind="ExternalInput")
    h0t_d = nc.dram_tensor("h0t", [128, C], fp32, kind="ExternalInput")
    y_d = nc.dram_tensor("y_out", [t_total // YB, 128, YB * C], fp32,
                         kind="ExternalOutput")
    h_d = nc.dram_tensor("h_out", [128, C], fp32, kind="ExternalOutput")

    with tile.TileContext(nc) as tc:
        with (
            tc.tile_pool(name="const", bufs=1) as cpool,
            tc.tile_pool(name="work", bufs=2) as wpool,
            tc.tile_pool(name="psum", bufs=1, space="PSUM") as ppool,
        ):
            ur_sb = cpool.tile([128, KT * G], bf16, name="ur_sb")
            w_sb = cpool.tile([128, ET * G], bf16, name="w_sb")
            xw_ab = [cpool.tile([128, TBLK * GT * B], bf16, name=f"xw{w}")
                     for w in range(2)]
            h_sb = cpool.tile([128, C], bf16, name="h_sb")
            b0pg = cpool.tile([128, GT], fp32, name="b0pg")
            b1n_bf = cpool.tile([128, C], bf16, name="b1n_bf")
            ident = cpool.tile([128, 128], bf16, name="ident")
            make_identity(nc, ident[:, :])

            for k in range(KT):
                nc.sync.dma_start(out=ur_sb[:, G * k:G * (k + 1)],
                                  in_=ur_d[128 * k:128 * (k + 1), :])
            for e in range(ET):
                nc.sync.dma_start(out=w_sb[:, G * e:G * (e + 1)],
                                  in_=w_d[128 * e:128 * (e + 1), :])
            nc.sync.dma_start(out=b0pg[:, :], in_=b0pg_d[:, :])
            bstg = wpool.tile([128, C], fp32, tag="bstg", bufs=1)
            nc.sync.dma_start(out=bstg[:, :], in_=b1n_d[:, :])
            nc.vector.tensor_copy(b1n_bf[:, :], bstg[:, :])
            h0stg = wpool.tile([128, C], fp32, tag="h0stg", bufs=1)
            nc.sync.dma_start(out=h0stg[:, :], in_=h0t_d[:, :])
            nc.vector.tensor_copy(h_sb[:, :], h0stg[:, :])

            # ------------------------------------------------------------------
            p1_xet = {}

            def p1_gather(win, t0, i):
                """Gather + transpose tok-tile i (of 4) for the next block of
                window `win` -> p1_xet[win]. Touches only xet, so it is safe
                while the window's CURRENT block is still being consumed."""
                if i == 0:
                    p1_xet[win] = wpool.tile(
                        [128, ET * TOKB], bf16, tag="xet", bufs=2, name="xet")
                xet = p1_xet[win]
                idx = wpool.tile([128, 1], i32, tag="idx", bufs=2, name="idx")
                nc.gpsimd.dma_start(out=idx[:, :],
                                    in_=x_d[ds(t0 + 4 * i, 4), :, :])
                # bounce via Pool compute: collapses the indirect DMA's deps
                # (idx ready + WAR on xe, prev reader also Pool) to ONE Pool
                # sem — dynamic DMAs fit a single sem wait
                idx2 = wpool.tile([128, 1], i32, tag="idx2", bufs=2,
                                  name="idx2")
                nc.gpsimd.tensor_copy(idx2[:, :], idx[:, :])
                xe = wpool.tile([128, E], fp32, tag="xe", bufs=2, name="xe")
                nc.gpsimd.indirect_dma_start(
                    out=xe[:, :], out_offset=None, in_=emb_d[:, :],
                    in_offset=bass.IndirectOffsetOnAxis(ap=idx2[:, :1], axis=0))
                # cast on Pool so the PE transpose sees ONE producer (the
                # LDWEIGHTS slot also fits a single sem wait)
                xeb = wpool.tile([128, E], bf16, tag="xeb", bufs=2, name="xeb")
                nc.gpsimd.tensor_copy(xeb[:, :], xe[:, :])
                for e in range(ET):
                    tp = ppool.tile([128, 128], bf16, tag="p1ps", bufs=2,
                                    name="tp")
                    nc.tensor.transpose(
                        out=tp[:, :], in_=xeb[:, 128 * e:128 * (e + 1)],
                        identity=ident[:, :])
                    nc.vector.tensor_copy(
                        xet[:, TOKB * e + 128 * i: TOKB * e + 128 * (i + 1)],
                        tp[:, :])

            def p1_mm(win, xw, j):
                """g-tile j of the xW matmul for window `win`'s next block.
                Writes xw columns of EVERY slab — only legal once the window's
                current block has been fully consumed."""
                xet = p1_xet[win]
                ps = ppool.tile([128, TOKB], fp32, tag="p1ps", bufs=2,
                                name="p1mm")
                for e in range(ET):
                    nc.tensor.matmul(
                        ps[:, :],
                        lhsT=w_sb[:, G * e + 128 * j: G * e + 128 * (j + 1)],
                        rhs=xet[:, TOKB * e: TOKB * (e + 1)],
                        start=(e == 0), stop=(e == ET - 1))
                src = ps[:, :].rearrange("p (t b) -> p t b", b=B)
                dst = xw[:, :].rearrange(
                    "p (t g b) -> p t g b", g=GT, b=B)[:, :, j, :]
                if j % 2 == 0:
                    nc.vector.tensor_scalar_add(dst, src, b0pg[:, j:j + 1])
                else:
                    nc.scalar.activation(dst, src, AF.Identity,
                                         bias=b0pg[:, j:j + 1])

            def phase1_block(win, t0, xw):
                for i in range(TOKB // 128):
                    p1_gather(win, t0, i)
                for j in range(GT):
                    p1_mm(win, xw, j)

            # ------------------------------------------------------------------
            yblk_cur = [None]
            last_hf = [None]

            def step(qbase, xw, s):
                """One recurrence step; consumes xw slab s, updates h_sb,
                stages y per YB steps. qbase = (time of slab 0) // YB."""
                xw0 = (GT * B) * s

                # G = h @ U_r via 24x8 (LDW + N=32 MM) pairs (~34ns each).
                # An identity matmul OPENS each PSUM region preloaded with the
                # precomputed xw slab (resp. recurrent bias b1_n), and the U_r
                # matmuls accumulate on top — the nonlinearity inputs come
                # straight from PSUM with no DVE adds. Three 1-bank PSUM
                # tiles, computed n -> r -> z, so the serial gate chain starts
                # as early as possible and overlaps the z-group matmuls.
                n_ps = ppool.tile([128, C], fp32, tag="n_ps", bufs=2,
                                  name="n_ps")
                r_ps = ppool.tile([128, C], fp32, tag="r_ps", bufs=2,
                                  name="r_ps")
                z_ps = ppool.tile([128, C], fp32, tag="z_ps", bufs=2,
                                  name="z_ps")

                def mm_group(ps, j0, opener_rhs):
                    nc.tensor.matmul(
                        ps[:, :], lhsT=ident[:, :], rhs=opener_rhs,
                        start=True, stop=False, skip_group_check=True)
                    for jj in range(8):
                        j = j0 + jj
                        for k in range(KT):
                            nc.tensor.matmul(
                                ps[:, 32 * jj:32 * (jj + 1)],
                                lhsT=ur_sb[:, G * k + 128 * j: G * k + 128 * (j + 1)],
                                rhs=h_sb[:, 32 * k:32 * (k + 1)],
                                start=False, stop=(jj == 7 and k == KT - 1),
                                skip_group_check=True)

                mm_group(n_ps, 16, b1n_bf[:, :])
                mm_group(r_ps, 8, xw[:, xw0 + C:xw0 + 2 * C])
                mm_group(z_ps, 0, xw[:, xw0:xw0 + C])

                # ---- gates ----
                rt = wpool.tile([128, C], bf16, tag="rt", bufs=2, name="rt")
                nc.scalar.activation(rt[:, :], r_ps[:, :], AF.Sigmoid)
                t2 = wpool.tile([128, C], bf16, tag="t2", bufs=2, name="t2")
                nc.vector.tensor_tensor(
                    out=t2[:, :], in0=n_ps[:, :], in1=rt[:, :], op=OP.mult)
                t3 = wpool.tile([128, C], bf16, tag="t3", bufs=2, name="t3")
                nc.vector.tensor_tensor(
                    out=t3[:, :], in0=t2[:, :],
                    in1=xw[:, xw0 + 2 * C:xw0 + 3 * C], op=OP.add)
                nt = wpool.tile([128, C], bf16, tag="nt", bufs=2, name="nt")
                nc.scalar.activation(nt[:, :], t3[:, :], AF.Tanh)
                dt_ = wpool.tile([128, C], bf16, tag="dt", bufs=2, name="dt")
                nc.vector.tensor_sub(dt_[:, :], h_sb[:, :], nt[:, :])
                zt = wpool.tile([128, C], bf16, tag="zt", bufs=2, name="zt")
                nc.scalar.activation(zt[:, :], z_ps[:, :], AF.Sigmoid)
                hf = wpool.tile([128, C], bf16, tag="hf", bufs=2, name="hf")
                nc.vector.tensor_mul(hf[:, :], zt[:, :], dt_[:, :])
                # final add writes h_sb directly (next step's MM operand);
                # the fp32 y staging copy happens off the critical path
                nc.vector.tensor_tensor(out=h_sb[:, :], in0=hf[:, :],
                                        in1=nt[:, :], op=OP.add)
                if s % YB == 0:
                    yblk_cur[0] = wpool.tile([128, YB * C], fp32, tag="yblk",
                                             bufs=2, name="yblk")
                yblk = yblk_cur[0]
                hf2 = yblk[:, (s % YB) * C:(s % YB + 1) * C]
                nc.vector.tensor_copy(hf2, h_sb[:, :])
                if s % YB == YB - 1:
                    dma_eng = [nc.sync, nc.scalar][(s // YB) % 2]
                    dma_eng.dma_start(out=y_d[ds(qbase + s // YB, 1), :, :],
                                      in_=yblk[:, :])
                last_hf[0] = hf2

            # ------------------------------------------------------------------
            phase1_block(0, 0, xw_ab[0])
            phase1_block(1, TBLK, xw_ab[1])

            if nblk > 2:
                with tc.For_i(0, (nblk - 2) * QB, 2 * QB) as q0:
                    # A-steps: filler = next-A gathers (xet only — xw_ab[0]
                    # is still being consumed)
                    for s in range(TBLK):
                        step(q0, xw_ab[0], s)
                        if s < TOKB // 128:
                            p1_gather(0, q0 * YB + 2 * TBLK, s)
                    # B-steps: filler = next-A matmuls (xw_ab[0] free now)
                    # + next-B gathers
                    for s in range(TBLK):
                        step(q0 + QB, xw_ab[1], s)
                        if s < TOKB // 128:
                            p1_gather(1, q0 * YB + 3 * TBLK, s)
                        for j in range((24 * s) // TBLK, (24 * (s + 1)) // TBLK):
                            p1_mm(0, xw_ab[0], j)
                    # next-B matmuls: one dense PE lump at the body tail
                    # (keeps HAM warm across the back-edge)
                    for j in range(GT):
                        p1_mm(1, xw_ab[1], j)
            te = (nblk - 2) * TBLK
            for s in range(TBLK):
                step(te // YB, xw_ab[0], s)
            for s in range(TBLK):
                step(te // YB + QB, xw_ab[1], s)

            nc.sync.dma_start(out=h_d[:, :], in_=last_hf[0])

    nc.finalize()
    return nc


# ----------------------------------------------------------------------------
# host-side packing / unpacking
# ----------------------------------------------------------------------------

def _pack_dir(x_tb, w, ur, b, h0, emb):
    import ml_dtypes
    b = np.asarray(b, np.float32)
    b0, b1 = b[0], b[1]
    badd = b0 + np.where(np.arange(G) < 2 * U, b1, 0.0)
    b0pg = np.ascontiguousarray(badd.reshape(GT, 128).T)
    b1n = np.ascontiguousarray(np.broadcast_to(
        b1[2 * U:].reshape(KT, 128).T[:, :, None], (128, KT, B)
    ).reshape(128, KT * B))
    h0t = np.ascontiguousarray(
        np.asarray(h0, np.float32).reshape(B, KT, 128).transpose(2, 1, 0)
    ).reshape(128, KT * B)
    return {
        "x_ids": np.ascontiguousarray(x_tb, np.int32).reshape(T, B, 1),
        "emb": np.asarray(emb, np.float32),
        "w": np.asarray(w, np.float32).astype(ml_dtypes.bfloat16),
        "ur": np.asarray(ur, np.float32).astype(ml_dtypes.bfloat16),
        "b0pg": b0pg, "b1n": b1n, "h0t": h0t,
    }


def _unpack_y(y_raw):
    """y_raw [T//4, 128, 4*KT*B] -> y [B, T, U];
    y_raw[q, p, 256*s + 32*k + b] = y[b, 4q+s, 128k+p]."""
    return np.ascontiguousarray(
        y_raw.reshape(T // YB, 128, YB, KT, B).transpose(4, 0, 2, 3, 1)
    ).reshape(B, T, U)


def _unpack_h(h_raw):
    return np.ascontiguousarray(
        h_raw.reshape(128, KT, B).transpose(2, 1, 0)
    ).reshape(B, U)


_CACHED = {}


def _get_program():
    if "nc" not in _CACHED:
        _CACHED["nc"] = build_program()
    return _CACHED["nc"]


def kernel(x, emb, W_f, U_f, b_f, W_b, U_b, b_b, h0_f, h0_b, _trace=False):
    from concourse.bass_utils import run_bass_kernel_spmd

    x = np.asarray(x)
    im_f = _pack_dir(x.T, W_f, U_f, b_f, h0_f, emb)
    im_b = _pack_dir(x.T[::-1], W_b, U_b, b_b, h0_b, emb)

    nc = _get_program()
    res = run_bass_kernel_spmd(
        nc, [im_f if c % 2 == 0 else im_b for c in range(N_CORES)],
        core_ids=list(range(N_CORES)), trace=_trace,
    )
    rf, rb = res.results[0], res.results[1]
    y_f = _unpack_y(rf["y_out"])
    y_b = _unpack_y(rb["y_out"])[:, ::-1]
    h_f = _unpack_h(rf["h_out"])
    h_b = _unpack_h(rb["h_out"])
    y = np.concatenate([y_f, y_b], axis=-1)
    if _trace:
        kernel.last_exec_ns = res.exec_time_ns
        kernel.last_results = res
    return (np.ascontiguousarray(y, np.float32), h_f.astype(np.float32),
            h_b.astype(np.float32))
